# revision 12
# baseline (speedup 1.0000x reference)
"""DeltaNet forward (nn_DeltaNet_31877247271472) as a Trainium2 Bass/Tile kernel.

Sharding: 8 NeuronCores, core c owns batch b=c//4 and heads {2*(c%4), 2*(c%4)+1}.
Each core computes its two heads' full pipeline (projections + causal conv/SiLU +
chunkwise delta rule + gating/RMSNorm + partial output projection); the host sums
the 4 partial outputs per batch.

Per-core algorithm (CHUNK=128, exact restructuring of the reference):
  - projections on the PE in fp32r (hidden pre-transposed to [D, L] on host)
  - delta-rule internals (Gram matrices, UT-transform inverse via the squaring
    series T=(I+B)(I+B^2)...(I+B^64), chunkwise scan) use bf16 operands with
    fp32 accumulation; validated to absmax_rel ~ 8.5e-4 vs the fp32 reference.
  - gating, RMSNorm and the final output projection accumulate in fp32/fp32r.
"""

import sys
from contextlib import ExitStack

try:
    import concourse.bass as bass  # noqa: F401
except ImportError:  # pragma: no cover - environment fallback
    sys.path.insert(0, "/opt/trn_rl_repo")

import numpy as np
import ml_dtypes

import concourse.bass as bass
import concourse.mybir as mybir
import concourse.tile as tile
from concourse import bacc
from concourse import bass_utils

F32 = mybir.dt.float32
F32R = mybir.dt.float32r
BF16 = mybir.dt.bfloat16
AF = mybir.ActivationFunctionType
ALU = mybir.AluOpType

N_CORES = 8
B, L, D, H = 2, 4096, 1024, 8
DK = DV = 128
CONV_K = 4
CHUNK = 128
NCH = L // CHUNK          # 32 chunks per head
KS = D // 128             # 8 contraction slices
LT = 512                  # L-tile (tokens per projection tile)
NLT = L // LT             # 8 L-tiles
CPT = LT // CHUNK         # 4 chunks per L-tile
EPS = 1e-5
STAGE = 99
SUB = 9  # debug bisect: 1=proj,2=conv,3=norms,4=transposes,5=gram+series,6=scan+gate,7=outproj


def build_program(n_ltiles=NLT):
    nc = bacc.Bacc("TRN2", target_bir_lowering=False, debug=False,
                   num_devices=N_CORES)

    dt = {}
    def din(name, shape, dtype):
        dt[name] = nc.dram_tensor(name, shape, dtype, kind="ExternalInput").ap()
    din("hT", (D, L), F32R)
    din("wq", (D, 2 * DK), F32R)
    din("wk", (D, 2 * DK), F32R)
    din("wv", (D, 2 * DV), F32R)
    din("wbm", (D, 4), F32R)
    din("bmb", (4, 1), F32)
    din("cwq", (2 * DK, CONV_K), F32)
    din("cwk", (2 * DK, CONV_K), F32)
    din("cwv", (2 * DV, CONV_K), F32)
    din("wo", (2 * DV, D), F32R)
    din("onw", (128, DV), F32)
    din("identf", (128, 128), F32)
    din("identb", (128, 128), BF16)
    din("masksl", (128, 128), BF16)
    din("maskui", (128, 128), BF16)
    din("onesc", (128, 1), BF16)
    din("onesr", (1, 128), BF16)
    out = nc.dram_tensor("out", (L, D), F32, kind="ExternalOutput").ap()

    with tile.TileContext(nc) as tc:
        with ExitStack() as ctx:
            _body(nc, tc, ctx, dt, out, n_ltiles)

    nc.compile()
    return nc


def _body(nc, tc, ctx, dt, out, n_ltiles=NLT):
    cpool = ctx.enter_context(tc.tile_pool(name="consts", bufs=1))
    hpool = ctx.enter_context(tc.tile_pool(name="hts", bufs=2))
    ppool = ctx.enter_context(tc.tile_pool(name="ptmp", bufs=2))
    projp = [ctx.enter_context(tc.tile_pool(name=f"proj{h}", bufs=3))
             for h in range(2)]
    dpool = ctx.enter_context(tc.tile_pool(name="delta", bufs=3))
    spool = ctx.enter_context(tc.tile_pool(name="series", bufs=2))
    opool = ctx.enter_context(tc.tile_pool(name="outb", bufs=2))
    # One PSUM pool; tag budget (1 bank per slot, 8 total):
    #   pp x2, psml x1, pmm x2, pacc x2, pout x1
    psum = ctx.enter_context(tc.tile_pool(name="psum", bufs=1, space="PSUM"))

    # ---- persistent weights/constants -------------------------------------
    wqs = cpool.tile([128, KS * 256], F32R)
    wks = cpool.tile([128, KS * 256], F32R)
    wvs = cpool.tile([128, KS * 256], F32R)
    wbms = cpool.tile([128, KS * 4], F32R)
    bmbias = cpool.tile([4, 1], F32)
    cwt = [[cpool.tile([128, CONV_K], F32, name=f"cw{p}{h}", tag=f"cw{p}{h}")
            for h in range(2)] for p in range(3)]
    wos = cpool.tile([128, 2 * D], F32R)
    onws = cpool.tile([128, DV], F32)
    identf = cpool.tile([128, 128], F32)
    identb = cpool.tile([128, 128], BF16)
    masksl = cpool.tile([128, 128], BF16)
    maskui = cpool.tile([128, 128], BF16)
    onesc = cpool.tile([128, 1], BF16)
    onesr = cpool.tile([1, 128], BF16)
    eps12 = cpool.tile([1, 1], F32)
    nc.vector.memset(eps12[:], 1e-12)
    eps5 = cpool.tile([128, 1], F32)
    nc.vector.memset(eps5[:], EPS)

    for ks in range(KS):
        r = slice(ks * 128, (ks + 1) * 128)
        nc.sync.dma_start(wqs[:, ks * 256:(ks + 1) * 256], dt["wq"][r, :])
        nc.sync.dma_start(wks[:, ks * 256:(ks + 1) * 256], dt["wk"][r, :])
        nc.sync.dma_start(wvs[:, ks * 256:(ks + 1) * 256], dt["wv"][r, :])
        nc.sync.dma_start(wbms[:, ks * 4:(ks + 1) * 4], dt["wbm"][r, :])
    nc.sync.dma_start(bmbias[:], dt["bmb"][:])
    for p, cwn in enumerate(("cwq", "cwk", "cwv")):
        for h in range(2):
            nc.sync.dma_start(cwt[p][h][:], dt[cwn][h * 128:(h + 1) * 128, :])
    for h in range(2):
        nc.sync.dma_start(wos[:, h * D:(h + 1) * D],
                          dt["wo"][h * 128:(h + 1) * 128, :])
    for n, t_ in (("onw", onws), ("identf", identf), ("identb", identb),
                  ("masksl", masksl), ("maskui", maskui), ("onesc", onesc),
                  ("onesr", onesr)):
        nc.sync.dma_start(t_[:], dt[n][:])

    # ---- loop-carried state ------------------------------------------------
    S = [cpool.tile([DK, DV], F32, name=f"S{h}", tag=f"S{h}") for h in range(2)]
    Sb = [cpool.tile([DK, DV], BF16, name=f"Sb{h}", tag=f"Sb{h}")
          for h in range(2)]
    for h in range(2):
        nc.vector.memset(S[h][:], 0.0)
        nc.vector.memset(Sb[h][:], 0.0)

    tails = {}
    for p in range(3):
        for h in range(2):
            tl = cpool.tile([128, CONV_K - 1], F32, name=f"tl{p}{h}",
                            tag=f"tail{p}{h}", bufs=2)
            nc.vector.memset(tl[:], 0.0)
            tails[(p, h)] = tl

    win = [{} for _ in range(2)]

    for t in range(n_ltiles):
        lo = t * LT
        # ================= phase P: projections for L-tile t ================
        hts = []
        for ks in range(KS):
            ht = hpool.tile([128, LT], F32R, name=f"ht{ks}", tag=f"ht{ks}",
                            bufs=2)
            nc.sync.dma_start(ht[:], dt["hT"][ks * 128:(ks + 1) * 128,
                                              lo:lo + LT])
            hts.append(ht)

        # beta/mix projection (uses the single small psum slot first)
        psbm = psum.tile([4, LT], F32, name="psbm", tag="psml", bufs=1)
        for ks in range(KS):
            nc.tensor.matmul(psbm[:], wbms[:, ks * 4:(ks + 1) * 4], hts[ks][:],
                             start=(ks == 0), stop=(ks == KS - 1))
        bmT = ppool.tile([4, LT], F32, name="bmT", tag="bmT", bufs=2)
        nc.scalar.activation(bmT[:], psbm[:], AF.Sigmoid, bias=bmbias[:, 0:1])
        bmL = ppool.tile([128, CPT * 4], F32, name="bmL", tag="bmL", bufs=2)
        for st in range(CPT):
            pt = psum.tile([128, 4], F32, name="pbmt", tag="psml", bufs=1)
            nc.tensor.transpose(pt[:], bmT[0:4, st * 128:(st + 1) * 128],
                                identf[0:4, 0:4])
            nc.vector.tensor_copy(bmL[:, st * 4:(st + 1) * 4], pt[:])

        # q/k/v projections for both heads: 3 passes of 2 psum banks
        ys_tiles = {}
        for p, wsb in enumerate((wqs, wks, wvs)):
            for h in range(2):
                ps = psum.tile([128, LT], F32, name=f"pp{p}{h}", tag="pp",
                               bufs=2)
                for ks in range(KS):
                    nc.tensor.matmul(
                        ps[:],
                        wsb[:, ks * 256 + h * 128: ks * 256 + (h + 1) * 128],
                        hts[ks][:], start=(ks == 0), stop=(ks == KS - 1))
                # causal depthwise conv (K=4) + SiLU
                if STAGE < 2:
                    ys_tiles[(p, h)] = None
                    continue
                xcat = ppool.tile([128, LT + CONV_K - 1], F32, name="xcat",
                                  tag="xcat", bufs=2)
                nc.scalar.copy(xcat[:, CONV_K - 1:], ps[:])
                nc.vector.tensor_copy(xcat[:, 0:CONV_K - 1], tails[(p, h)][:])
                ntail = cpool.tile([128, CONV_K - 1], F32, name=f"ntl{p}{h}",
                                   tag=f"tail{p}{h}", bufs=2)
                nc.vector.tensor_copy(ntail[:], xcat[:, LT:LT + CONV_K - 1])
                tails[(p, h)] = ntail
                cw = cwt[p][h]
                ya = ppool.tile([128, LT], F32, name="ya", tag="ya", bufs=2)
                yb = ppool.tile([128, LT], F32, name="yb", tag="yb", bufs=2)
                nc.vector.tensor_scalar_mul(ya[:], xcat[:, 3:3 + LT],
                                            cw[:, 3:4])
                nc.vector.scalar_tensor_tensor(
                    yb[:], xcat[:, 2:2 + LT], cw[:, 2:3], ya[:],
                    ALU.mult, ALU.add)
                nc.vector.scalar_tensor_tensor(
                    ya[:], xcat[:, 1:1 + LT], cw[:, 1:2], yb[:],
                    ALU.mult, ALU.add)
                nc.vector.scalar_tensor_tensor(
                    yb[:], xcat[:, 0:0 + LT], cw[:, 0:1], ya[:],
                    ALU.mult, ALU.add)
                # SiLU = x * sigmoid(x) (Silu ACT fn is not in CoreSim)
                sgm = ppool.tile([128, LT], F32, name="sgm", tag="sgm", bufs=2)
                nc.scalar.activation(sgm[:], yb[:], AF.Sigmoid)
                ys = ppool.tile([128, LT], F32, name=f"ys{p}{h}",
                                tag=f"ys{p}{h}", bufs=2)
                nc.vector.tensor_mul(ys[:], yb[:], sgm[:])
                ys_tiles[(p, h)] = ys

        if STAGE < 3:
            continue
        for h in range(2):
            w = win[h]
            # q,k: token-wise l2 norm via column-sum matmul trick (d-major)
            for p, name in ((0, "qTb"), (1, "kTb")):
                ys = ys_tiles[(p, h)]
                sq = ppool.tile([128, LT], BF16, name="sqt", tag="sqt", bufs=2)
                nc.vector.tensor_mul(sq[:], ys[:], ys[:])
                prow = psum.tile([1, LT], F32, name="prow", tag="psml", bufs=1)
                nc.tensor.matmul(prow[:], onesc[:], sq[:], start=True,
                                 stop=True)
                rrow = ppool.tile([1, LT], F32, name="rrow", tag="rrow",
                                  bufs=2)
                nc.scalar.activation(rrow[:], prow[:], AF.Sqrt,
                                     bias=eps12[:, 0:1])
                nc.vector.reciprocal(rrow[:], rrow[:])
                rrowb = ppool.tile([1, LT], BF16, name="rrowb", tag="rrowb",
                                   bufs=2)
                nc.vector.tensor_copy(rrowb[:], rrow[:])
                prep = psum.tile([128, LT], F32, name="prep", tag="psml",
                                 bufs=1)
                nc.tensor.matmul(prep[:], onesr[:], rrowb[:], start=True,
                                 stop=True)
                nT = projp[h].tile([128, LT], BF16, name=name, tag=name)
                nc.vector.tensor_mul(nT[:], ys[:], prep[:])
                w[(name, t)] = nT

            if STAGE < 4:
                continue
            # k -> l-major (normalized) and kb = beta*k
            kT = w[("kTb", t)]
            kL = projp[h].tile([128, LT], BF16, name="kL", tag="kL")
            kbL = projp[h].tile([128, LT], BF16, name="kbL", tag="kbL")
            for st in range(CPT):
                ptr = psum.tile([128, 128], BF16, name="ptr", tag="psml",
                                bufs=1)
                nc.tensor.transpose(ptr[:], kT[:, st * 128:(st + 1) * 128],
                                    identb[:])
                nc.vector.tensor_copy(kL[:, st * 128:(st + 1) * 128], ptr[:])
                nc.vector.tensor_scalar_mul(
                    kbL[:, st * 128:(st + 1) * 128], ptr[:],
                    bmL[:, st * 4 + h: st * 4 + h + 1])
            w[("kL", t)] = kL
            w[("kbL", t)] = kbL

            # v -> l-major, vb = beta*v
            vs = ys_tiles[(2, h)]
            vLb = projp[h].tile([128, LT], BF16, name="vLb", tag="vLb")
            vbL = projp[h].tile([128, LT], BF16, name="vbL", tag="vbL")
            for st in range(CPT):
                ptr = psum.tile([128, 128], F32, name="ptrv", tag="psml",
                                bufs=1)
                nc.tensor.transpose(ptr[:], vs[:, st * 128:(st + 1) * 128],
                                    identf[:])
                nc.vector.tensor_copy(vLb[:, st * 128:(st + 1) * 128], ptr[:])
                nc.vector.tensor_scalar_mul(
                    vbL[:, st * 128:(st + 1) * 128], ptr[:],
                    bmL[:, st * 4 + h: st * 4 + h + 1])
            w[("vLb", t)] = vLb
            w[("vbL", t)] = vbL
            w[("bmL", t)] = bmL

        # ================= phase D: delta rule, 4 chunks ====================
        if STAGE < 5:
            continue
        for cc in range(CPT):
            c = t * CPT + cc
            cs = slice(cc * 128, (cc + 1) * 128)
            ogTs = []
            for h in range(2):
                w = win[h]
                qT, kT = w[("qTb", t)], w[("kTb", t)]
                kL, kbL = w[("kL", t)], w[("kbL", t)]
                vLb, vbL = w[("vLb", t)], w[("vbL", t)]
                bml = w[("bmL", t)]
                beta_col = bml[:, cc * 4 + h: cc * 4 + h + 1]
                g_col = bml[:, cc * 4 + 2 + h: cc * 4 + 3 + h]

                # Gram matrices
                pg = psum.tile([128, 128], F32, name="pg", tag="pmm", bufs=2)
                nc.tensor.matmul(pg[:], kT[:, cs], kT[:, cs], start=True,
                                 stop=True)
                pa = psum.tile([128, 128], F32, name="pa", tag="pmm", bufs=2)
                nc.tensor.matmul(pa[:], kT[:, cs], qT[:, cs], start=True,
                                 stop=True)
                AN = dpool.tile([128, 128], BF16, name="AN", tag="AN")
                nc.vector.scalar_tensor_tensor(AN[:], pg[:], beta_col,
                                               masksl[:], ALU.mult, ALU.mult)
                attnT = dpool.tile([128, 128], BF16, name="attnT", tag="attnT")
                nc.vector.tensor_mul(attnT[:], pa[:], maskui[:])
                pat = psum.tile([128, 128], BF16, name="pat", tag="pmm",
                                bufs=2)
                nc.tensor.transpose(pat[:], AN[:], identb[:])
                ATl = dpool.tile([128, 128], BF16, name="ATl", tag="ATl")
                nc.vector.tensor_copy(ATl[:], pat[:])

                # UT-transform inverse via squaring series (bf16 operands)
                MTs = spool.tile([128, 128], BF16, name="MTs", tag="MTs")
                nc.vector.tensor_add(MTs[:], ATl[:], identb[:])
                BN, BT = AN, ATl
                for j in range(1, 7):
                    pbn = psum.tile([128, 128], F32, name="pbn", tag="pmm",
                                    bufs=2)
                    nc.tensor.matmul(pbn[:], BT[:], BN[:], start=True,
                                     stop=True)
                    BN2 = spool.tile([128, 128], BF16, name="BN2", tag="BN2")
                    nc.vector.tensor_copy(BN2[:], pbn[:])
                    if j < 6:
                        pbt = psum.tile([128, 128], F32, name="pbt", tag="pmm",
                                        bufs=2)
                        nc.tensor.matmul(pbt[:], BN[:], BT[:], start=True,
                                         stop=True)
                        BT2 = spool.tile([128, 128], BF16, name="BT2",
                                         tag="BT2")
                        nc.scalar.copy(BT2[:], pbt[:])
                    else:
                        BT2 = None
                    pmt = psum.tile([128, 128], F32, name="pmt", tag="pacc",
                                    bufs=2)
                    nc.tensor.matmul(pmt[:], BN2[:], MTs[:], start=True,
                                     stop=True)
                    MTn = spool.tile([128, 128], BF16, name="MTn", tag="MTs")
                    nc.vector.tensor_add(MTn[:], MTs[:], pmt[:])
                    MTs = MTn
                    BN, BT = BN2, BT2
                TT = MTs

                if STAGE < 6:
                    ogTs.append(None)
                    continue
                # u = T @ (beta v),   wT = -(T @ (beta k))^T
                pu = psum.tile([128, 128], F32, name="pu", tag="pmm", bufs=2)
                nc.tensor.matmul(pu[:], TT[:], vbL[:, cs], start=True,
                                 stop=True)
                uL = dpool.tile([128, 128], BF16, name="uL", tag="uL")
                nc.vector.tensor_copy(uL[:], pu[:])
                pw = psum.tile([128, 128], F32, name="pw", tag="pmm", bufs=2)
                nc.tensor.matmul(pw[:], kbL[:, cs], TT[:], start=True,
                                 stop=True)
                wTs = dpool.tile([128, 128], BF16, name="wTs", tag="wTs")
                nc.vector.tensor_scalar_mul(wTs[:], pw[:], -1.0)

                # ---- sequential scan step ----
                if SUB < 2:
                    ogTs.append(None)
                    continue
                pup = psum.tile([128, 128], F32, name="pup", tag="pmm",
                                bufs=2)
                nc.tensor.matmul(pup[:], wTs[:], Sb[h][:], start=True,
                                 stop=True)
                upb = dpool.tile([128, 128], BF16, name="upb", tag="upb")
                nc.vector.tensor_add(upb[:], uL[:], pup[:])
                if SUB < 3:
                    ogTs.append(None)
                    continue
                po = psum.tile([128, 128], F32, name="po", tag="pacc", bufs=2)
                nc.tensor.matmul(po[:], qT[:, cs], Sb[h][:], start=True,
                                 stop=False)
                nc.tensor.matmul(po[:], attnT[:], upb[:], start=False,
                                 stop=True)
                if SUB < 4:
                    ogTs.append(None)
                    continue
                pds = psum.tile([128, 128], F32, name="pds", tag="pmm",
                                bufs=2)
                nc.tensor.matmul(pds[:], kL[:, cs], upb[:], start=True,
                                 stop=True)
                nc.vector.tensor_add(S[h][:], S[h][:], pds[:])
                nc.scalar.copy(Sb[h][:], S[h][:])

                # ---- gating + per-head RMSNorm + transpose ----
                if SUB < 5:
                    ogTs.append(None)
                    continue
                og = dpool.tile([128, 128], F32, name="og", tag="og")
                nc.vector.tensor_sub(og[:], po[:], vLb[:, cs])
                og2 = dpool.tile([128, 128], F32, name="og2", tag="og2")
                nc.vector.scalar_tensor_tensor(og2[:], og[:], g_col,
                                               vLb[:, cs], ALU.mult, ALU.add)
                scr = dpool.tile([128, 128], F32, name="scr", tag="scr")
                ssq = dpool.tile([128, 1], F32, name="ssq", tag="ssq")
                nc.scalar.activation(scr[:], og2[:], AF.Square,
                                     accum_out=ssq[:])
                nr = dpool.tile([128, 1], F32, name="nr", tag="nr")
                nc.scalar.activation(nr[:], ssq[:], AF.Sqrt,
                                     bias=eps5[:, 0:1], scale=1.0 / DV)
                nc.vector.reciprocal(nr[:], nr[:])
                ogn = dpool.tile([128, 128], F32, name="ogn", tag="ogn")
                nc.vector.scalar_tensor_tensor(ogn[:], og2[:], nr[:, 0:1],
                                               onws[:], ALU.mult, ALU.mult)
                if SUB < 6:
                    ogTs.append(None)
                    continue
                pogt = psum.tile([128, 128], F32, name="pogt", tag="pmm",
                                 bufs=2)
                nc.tensor.transpose(pogt[:], ogn[:], identf[:])
                ogT = dpool.tile([128, 128], F32R, name="ogT", tag="ogT")
                nc.vector.tensor_copy(ogT[:], pogt[:])
                ogTs.append(ogT)

            # ---- output projection for chunk c (both heads accumulated) ----
            if STAGE < 7:
                continue
            outb = opool.tile([128, D], F32, name="outb", tag="outb")
            for half in range(2):
                pout = psum.tile([128, 512], F32, name="pout", tag="pout",
                                 bufs=1)
                for h in range(2):
                    nc.tensor.matmul(
                        pout[:], ogTs[h][:],
                        wos[:, h * D + half * 512: h * D + (half + 1) * 512],
                        start=(h == 0), stop=(h == 1))
                nc.scalar.copy(outb[:, half * 512:(half + 1) * 512], pout[:])
            nc.sync.dma_start(out[c * 128:(c + 1) * 128, :], outb[:])


_NC_CACHE = None


def _get_program():
    global _NC_CACHE
    if _NC_CACHE is None:
        _NC_CACHE = build_program()
    return _NC_CACHE


def _make_consts():
    bf = ml_dtypes.bfloat16
    ident = np.eye(128, dtype=np.float32)
    return {
        "identf": ident,
        "identb": ident.astype(bf),
        "masksl": (np.tril(np.ones((128, 128), np.float32), -1) * -1.0).astype(bf),
        "maskui": np.triu(np.ones((128, 128), np.float32)).astype(bf),
        "onesc": np.ones((128, 1), np.float32).astype(bf),
        "onesr": np.ones((1, 128), np.float32).astype(bf),
    }


def make_in_maps(inputs):
    hidden = np.asarray(inputs["hidden_states"], np.float32)
    q_w = np.asarray(inputs["q_w"], np.float32)
    k_w = np.asarray(inputs["k_w"], np.float32)
    v_w = np.asarray(inputs["v_w"], np.float32)
    conv_q = np.asarray(inputs["conv_q_w"], np.float32)
    conv_k = np.asarray(inputs["conv_k_w"], np.float32)
    conv_v = np.asarray(inputs["conv_v_w"], np.float32)
    b_w = np.asarray(inputs["b_w"], np.float32)
    mix_w = np.asarray(inputs["mix_w"], np.float32)
    mix_b = np.asarray(inputs["mix_b"], np.float32)
    mix_bias = np.asarray(inputs["mix_bias"], np.float32)
    o_norm_w = np.asarray(inputs["o_norm_w"], np.float32)
    o_w = np.asarray(inputs["o_w"], np.float32)

    consts = _make_consts()
    hT_by_batch = [np.ascontiguousarray(hidden[b].T) for b in range(B)]
    onw_rep = np.ascontiguousarray(np.tile(o_norm_w[None, :], (128, 1)))

    in_maps = []
    for c in range(N_CORES):
        b = c // 4
        h0 = 2 * (c % 4)
        hsl = slice(h0 * DK, (h0 + 2) * DK)
        wbm = np.ascontiguousarray(
            np.stack([b_w[:, h0], b_w[:, h0 + 1],
                      mix_w[:, h0], mix_w[:, h0 + 1]], axis=1))
        bmbias = np.array([[0.0], [0.0],
                           [mix_b[h0] + mix_bias[h0]],
                           [mix_b[h0 + 1] + mix_bias[h0 + 1]]], np.float32)
        m = {
            "hT": hT_by_batch[b],
            "wq": np.ascontiguousarray(q_w[:, hsl]),
            "wk": np.ascontiguousarray(k_w[:, hsl]),
            "wv": np.ascontiguousarray(v_w[:, hsl]),
            "wbm": wbm,
            "bmb": bmbias,
            "cwq": np.ascontiguousarray(conv_q[hsl, :]),
            "cwk": np.ascontiguousarray(conv_k[hsl, :]),
            "cwv": np.ascontiguousarray(conv_v[hsl, :]),
            "wo": np.ascontiguousarray(o_w[hsl, :]),
            "onw": onw_rep,
        }
        m.update(consts)
        in_maps.append(m)
    return in_maps


def kernel(**inputs):
    nc = _get_program()
    in_maps = make_in_maps(inputs)
    res = bass_utils.run_bass_kernel_spmd(nc, in_maps,
                                          core_ids=list(range(N_CORES)))
    outp = np.zeros((B, L, D), np.float32)
    for c in range(N_CORES):
        outp[c // 4] += res.results[c]["out"]
    return outp


# revision 15
# speedup vs baseline: 1.1502x; 1.1502x over previous
"""DeltaNet forward (nn_DeltaNet_31877247271472) as a Trainium2 Bass/Tile kernel.

Sharding: 8 NeuronCores, core c owns batch b=c//4 and heads {2*(c%4), 2*(c%4)+1}.
Each core computes its two heads' full pipeline (projections + causal conv/SiLU +
chunkwise delta rule + gating/RMSNorm + partial output projection); the host sums
the 4 partial outputs per batch.

Per-core algorithm (CHUNK=128, exact restructuring of the reference):
  - projections on the PE in fp32r (hidden pre-transposed to [D, L] on host)
  - delta-rule internals (Gram matrices, UT-transform inverse via the squaring
    series T=(I+B)(I+B^2)...(I+B^64), chunkwise scan) use bf16 operands with
    fp32 accumulation; validated to absmax_rel ~ 8.5e-4 vs the fp32 reference.
  - gating, RMSNorm and the final output projection accumulate in fp32/fp32r.
"""

import sys
from contextlib import ExitStack

try:
    import concourse.bass as bass  # noqa: F401
except ImportError:  # pragma: no cover - environment fallback
    sys.path.insert(0, "/opt/trn_rl_repo")

import numpy as np
import ml_dtypes

import concourse.bass as bass
import concourse.mybir as mybir
import concourse.tile as tile
from concourse import bacc
from concourse import bass_utils

F32 = mybir.dt.float32
F32R = mybir.dt.float32r
BF16 = mybir.dt.bfloat16
AF = mybir.ActivationFunctionType
ALU = mybir.AluOpType

N_CORES = 8
B, L, D, H = 2, 4096, 1024, 8
DK = DV = 128
CONV_K = 4
CHUNK = 128
NCH = L // CHUNK          # 32 chunks per head
KS = D // 128             # 8 contraction slices
LT = 512                  # L-tile (tokens per projection tile)
NLT = L // LT             # 8 L-tiles
CPT = LT // CHUNK         # 4 chunks per L-tile
EPS = 1e-5
STAGE = 99
SUB = 9  # debug bisect
SIM_SAFE = False  # True: emulate SiLU via Sigmoid+mult for CoreSim: 1=proj,2=conv,3=norms,4=transposes,5=gram+series,6=scan+gate,7=outproj


def build_program(n_ltiles=NLT):
    nc = bacc.Bacc("TRN2", target_bir_lowering=False, debug=False,
                   num_devices=N_CORES)

    dt = {}
    def din(name, shape, dtype):
        dt[name] = nc.dram_tensor(name, shape, dtype, kind="ExternalInput").ap()
    din("hT", (D, L), F32R)
    din("wq", (D, 2 * DK), F32R)
    din("wk", (D, 2 * DK), F32R)
    din("wv", (D, 2 * DV), F32R)
    din("wbm", (D, 4), F32R)
    din("bmb", (4, 1), F32)
    din("cwq", (2 * DK, CONV_K), F32)
    din("cwk", (2 * DK, CONV_K), F32)
    din("cwv", (2 * DV, CONV_K), F32)
    din("wo", (2 * DV, D), F32R)
    din("onw", (128, DV), F32)
    din("identf", (128, 128), F32)
    din("identb", (128, 128), BF16)
    din("masksl", (128, 128), BF16)
    din("maskui", (128, 128), BF16)
    din("onesc", (128, 1), BF16)
    din("onesr", (1, 128), BF16)
    out = nc.dram_tensor("out", (L, D), F32, kind="ExternalOutput").ap()

    with tile.TileContext(nc) as tc:
        with ExitStack() as ctx:
            _body(nc, tc, ctx, dt, out, n_ltiles)

    nc.compile()
    return nc


def _body(nc, tc, ctx, dt, out, n_ltiles=NLT):
    cpool = ctx.enter_context(tc.tile_pool(name="consts", bufs=1))
    hpool = ctx.enter_context(tc.tile_pool(name="hts", bufs=2))
    ppool = ctx.enter_context(tc.tile_pool(name="ptmp", bufs=2))
    projp = [ctx.enter_context(tc.tile_pool(name=f"proj{h}", bufs=3))
             for h in range(2)]
    dpool = ctx.enter_context(tc.tile_pool(name="delta", bufs=3))
    spool = ctx.enter_context(tc.tile_pool(name="series", bufs=2))
    opool = ctx.enter_context(tc.tile_pool(name="outb", bufs=2))
    # One PSUM pool; tag budget (1 bank per slot, 8 total):
    #   pp x2, psml x1, pmm x2, pacc x2, pout x1
    psum = ctx.enter_context(tc.tile_pool(name="psum", bufs=1, space="PSUM"))

    # ---- persistent weights/constants -------------------------------------
    wqs = cpool.tile([128, KS * 256], F32R)
    wks = cpool.tile([128, KS * 256], F32R)
    wvs = cpool.tile([128, KS * 256], F32R)
    wbms = cpool.tile([128, KS * 4], F32R)
    bmbias = cpool.tile([4, 1], F32)
    cwt = [[cpool.tile([128, CONV_K], F32, name=f"cw{p}{h}", tag=f"cw{p}{h}")
            for h in range(2)] for p in range(3)]
    wos = cpool.tile([128, 2 * D], F32R)
    onws = cpool.tile([128, DV], F32)
    identf = cpool.tile([128, 128], F32)
    identb = cpool.tile([128, 128], BF16)
    masksl = cpool.tile([128, 128], BF16)
    maskui = cpool.tile([128, 128], BF16)
    onesc = cpool.tile([128, 1], BF16)
    onesr = cpool.tile([1, 128], BF16)
    eps12 = cpool.tile([1, 1], F32)
    nc.vector.memset(eps12[:], 1e-12)
    eps5 = cpool.tile([128, 1], F32)
    nc.vector.memset(eps5[:], EPS)

    for ks in range(KS):
        r = slice(ks * 128, (ks + 1) * 128)
        nc.sync.dma_start(wqs[:, ks * 256:(ks + 1) * 256], dt["wq"][r, :])
        nc.sync.dma_start(wks[:, ks * 256:(ks + 1) * 256], dt["wk"][r, :])
        nc.sync.dma_start(wvs[:, ks * 256:(ks + 1) * 256], dt["wv"][r, :])
        nc.sync.dma_start(wbms[:, ks * 4:(ks + 1) * 4], dt["wbm"][r, :])
    nc.sync.dma_start(bmbias[:], dt["bmb"][:])
    for p, cwn in enumerate(("cwq", "cwk", "cwv")):
        for h in range(2):
            nc.sync.dma_start(cwt[p][h][:], dt[cwn][h * 128:(h + 1) * 128, :])
    for h in range(2):
        nc.sync.dma_start(wos[:, h * D:(h + 1) * D],
                          dt["wo"][h * 128:(h + 1) * 128, :])
    for n, t_ in (("onw", onws), ("identf", identf), ("identb", identb),
                  ("masksl", masksl), ("maskui", maskui), ("onesc", onesc),
                  ("onesr", onesr)):
        nc.sync.dma_start(t_[:], dt[n][:])

    # ---- loop-carried state ------------------------------------------------
    S = [cpool.tile([DK, DV], F32, name=f"S{h}", tag=f"S{h}") for h in range(2)]
    Sb = [cpool.tile([DK, DV], BF16, name=f"Sb{h}", tag=f"Sb{h}")
          for h in range(2)]
    for h in range(2):
        nc.vector.memset(S[h][:], 0.0)
        nc.vector.memset(Sb[h][:], 0.0)

    tails = {}
    for p in range(3):
        for h in range(2):
            tl = cpool.tile([128, CONV_K - 1], F32, name=f"tl{p}{h}",
                            tag=f"tail{p}{h}", bufs=2)
            nc.vector.memset(tl[:], 0.0)
            tails[(p, h)] = tl

    win = [{} for _ in range(2)]

    for t in range(n_ltiles):
        lo = t * LT
        # ================= phase P: projections for L-tile t ================
        hts = []
        for ks in range(KS):
            ht = hpool.tile([128, LT], F32R, name=f"ht{ks}", tag=f"ht{ks}",
                            bufs=2)
            nc.sync.dma_start(ht[:], dt["hT"][ks * 128:(ks + 1) * 128,
                                              lo:lo + LT])
            hts.append(ht)

        # beta/mix projection (uses the single small psum slot first)
        psbm = psum.tile([4, LT], F32, name="psbm", tag="psml", bufs=1)
        for ks in range(KS):
            nc.tensor.matmul(psbm[:], wbms[:, ks * 4:(ks + 1) * 4], hts[ks][:],
                             start=(ks == 0), stop=(ks == KS - 1))
        bmT = ppool.tile([4, LT], F32, name="bmT", tag="bmT", bufs=2)
        nc.scalar.activation(bmT[:], psbm[:], AF.Sigmoid, bias=bmbias[:, 0:1])
        bmL = ppool.tile([128, CPT * 4], F32, name="bmL", tag="bmL", bufs=2)
        for st in range(CPT):
            pt = psum.tile([128, 4], F32, name="pbmt", tag="psml", bufs=1)
            nc.tensor.transpose(pt[:], bmT[0:4, st * 128:(st + 1) * 128],
                                identf[0:4, 0:4])
            nc.vector.tensor_copy(bmL[:, st * 4:(st + 1) * 4], pt[:])

        # q/k/v projections for both heads: 3 passes of 2 psum banks
        ys_tiles = {}
        for p, wsb in enumerate((wqs, wks, wvs)):
            for h in range(2):
                ps = psum.tile([128, LT], F32, name=f"pp{p}{h}", tag="pp",
                               bufs=2)
                for ks in range(KS):
                    nc.tensor.matmul(
                        ps[:],
                        wsb[:, ks * 256 + h * 128: ks * 256 + (h + 1) * 128],
                        hts[ks][:], start=(ks == 0), stop=(ks == KS - 1))
                # causal depthwise conv (K=4) + SiLU
                if STAGE < 2:
                    ys_tiles[(p, h)] = None
                    continue
                xcat = ppool.tile([128, LT + CONV_K - 1], F32, name="xcat",
                                  tag="xcat", bufs=2)
                nc.scalar.copy(xcat[:, CONV_K - 1:], ps[:])
                nc.vector.tensor_copy(xcat[:, 0:CONV_K - 1], tails[(p, h)][:])
                ntail = cpool.tile([128, CONV_K - 1], F32, name=f"ntl{p}{h}",
                                   tag=f"tail{p}{h}", bufs=2)
                nc.vector.tensor_copy(ntail[:], xcat[:, LT:LT + CONV_K - 1])
                tails[(p, h)] = ntail
                cw = cwt[p][h]
                ya = ppool.tile([128, LT], F32, name="ya", tag="ya", bufs=2)
                yb = ppool.tile([128, LT], F32, name="yb", tag="yb", bufs=2)
                nc.vector.tensor_scalar_mul(ya[:], xcat[:, 3:3 + LT],
                                            cw[:, 3:4])
                nc.vector.scalar_tensor_tensor(
                    yb[:], xcat[:, 2:2 + LT], cw[:, 2:3], ya[:],
                    ALU.mult, ALU.add)
                nc.vector.scalar_tensor_tensor(
                    ya[:], xcat[:, 1:1 + LT], cw[:, 1:2], yb[:],
                    ALU.mult, ALU.add)
                nc.vector.scalar_tensor_tensor(
                    yb[:], xcat[:, 0:0 + LT], cw[:, 0:1], ya[:],
                    ALU.mult, ALU.add)
                ys = ppool.tile([128, LT], F32, name=f"ys{p}{h}",
                                tag=f"ys{p}{h}", bufs=2)
                if SIM_SAFE:
                    # CoreSim has no Silu table; emulate
                    sgm = ppool.tile([128, LT], F32, name="sgm", tag="sgm",
                                     bufs=2)
                    nc.scalar.activation(sgm[:], yb[:], AF.Sigmoid)
                    nc.vector.tensor_mul(ys[:], yb[:], sgm[:])
                else:
                    nc.scalar.activation(ys[:], yb[:], AF.Silu)
                ys_tiles[(p, h)] = ys

        if STAGE < 3:
            continue
        for h in range(2):
            w = win[h]
            # q,k: token-wise l2 norm via column-sum matmul trick (d-major)
            for p, name in ((0, "qTb"), (1, "kTb")):
                ys = ys_tiles[(p, h)]
                sq = ppool.tile([128, LT], BF16, name="sqt", tag="sqt", bufs=2)
                nc.vector.tensor_mul(sq[:], ys[:], ys[:])
                prow = psum.tile([1, LT], F32, name="prow", tag="psml", bufs=1)
                nc.tensor.matmul(prow[:], onesc[:], sq[:], start=True,
                                 stop=True)
                rrow = ppool.tile([1, LT], F32, name="rrow", tag="rrow",
                                  bufs=2)
                nc.scalar.activation(rrow[:], prow[:], AF.Sqrt,
                                     bias=eps12[:, 0:1])
                nc.vector.reciprocal(rrow[:], rrow[:])
                rrowb = ppool.tile([1, LT], BF16, name="rrowb", tag="rrowb",
                                   bufs=2)
                nc.vector.tensor_copy(rrowb[:], rrow[:])
                prep = psum.tile([128, LT], F32, name="prep", tag="psml",
                                 bufs=1)
                nc.tensor.matmul(prep[:], onesr[:], rrowb[:], start=True,
                                 stop=True)
                nT = projp[h].tile([128, LT], BF16, name=name, tag=name)
                nc.vector.tensor_mul(nT[:], ys[:], prep[:])
                w[(name, t)] = nT

            if STAGE < 4:
                continue
            # k -> l-major (normalized) and kb = beta*k
            kT = w[("kTb", t)]
            kL = projp[h].tile([128, LT], BF16, name="kL", tag="kL")
            kbL = projp[h].tile([128, LT], BF16, name="kbL", tag="kbL")
            for st in range(CPT):
                ptr = psum.tile([128, 128], BF16, name="ptr", tag="psml",
                                bufs=1)
                nc.tensor.transpose(ptr[:], kT[:, st * 128:(st + 1) * 128],
                                    identb[:])
                nc.vector.tensor_copy(kL[:, st * 128:(st + 1) * 128], ptr[:])
                nc.vector.tensor_scalar_mul(
                    kbL[:, st * 128:(st + 1) * 128], ptr[:],
                    bmL[:, st * 4 + h: st * 4 + h + 1])
            w[("kL", t)] = kL
            w[("kbL", t)] = kbL

            # v -> l-major, vb = beta*v
            vs = ys_tiles[(2, h)]
            vLb = projp[h].tile([128, LT], BF16, name="vLb", tag="vLb")
            vbL = projp[h].tile([128, LT], BF16, name="vbL", tag="vbL")
            for st in range(CPT):
                ptr = psum.tile([128, 128], F32, name="ptrv", tag="psml",
                                bufs=1)
                nc.tensor.transpose(ptr[:], vs[:, st * 128:(st + 1) * 128],
                                    identf[:])
                nc.vector.tensor_copy(vLb[:, st * 128:(st + 1) * 128], ptr[:])
                nc.vector.tensor_scalar_mul(
                    vbL[:, st * 128:(st + 1) * 128], ptr[:],
                    bmL[:, st * 4 + h: st * 4 + h + 1])
            w[("vLb", t)] = vLb
            w[("vbL", t)] = vbL
            w[("bmL", t)] = bmL

        # ================= phase D: delta rule, 4 chunks ====================
        if STAGE < 5:
            continue
        og2s = {}
        for cc in range(CPT):
            c = t * CPT + cc
            cs = slice(cc * 128, (cc + 1) * 128)
            for h in range(2):
                w = win[h]
                qT, kT = w[("qTb", t)], w[("kTb", t)]
                kL, kbL = w[("kL", t)], w[("kbL", t)]
                vLb, vbL = w[("vLb", t)], w[("vbL", t)]
                bml = w[("bmL", t)]
                beta_col = bml[:, cc * 4 + h: cc * 4 + h + 1]
                g_col = bml[:, cc * 4 + 2 + h: cc * 4 + 3 + h]

                # Gram matrices
                pg = psum.tile([128, 128], F32, name="pg", tag="pmm", bufs=2)
                nc.tensor.matmul(pg[:], kT[:, cs], kT[:, cs], start=True,
                                 stop=True)
                pa = psum.tile([128, 128], F32, name="pa", tag="pmm", bufs=2)
                nc.tensor.matmul(pa[:], kT[:, cs], qT[:, cs], start=True,
                                 stop=True)
                AN = dpool.tile([128, 128], BF16, name="AN", tag="AN")
                nc.vector.scalar_tensor_tensor(AN[:], pg[:], beta_col,
                                               masksl[:], ALU.mult, ALU.mult)
                attnT = dpool.tile([128, 128], BF16, name="attnT", tag="attnT")
                nc.vector.tensor_mul(attnT[:], pa[:], maskui[:])
                pat = psum.tile([128, 128], BF16, name="pat", tag="pmm",
                                bufs=2)
                nc.tensor.transpose(pat[:], AN[:], identb[:])
                ATl = dpool.tile([128, 128], BF16, name="ATl", tag="ATl")
                nc.vector.tensor_copy(ATl[:], pat[:])

                # UT-transform inverse via squaring series (bf16 operands)
                MTs = spool.tile([128, 128], BF16, name="MTs", tag="MTs")
                nc.vector.tensor_add(MTs[:], ATl[:], identb[:])
                BN, BT = AN, ATl
                for j in range(1, 7):
                    pbn = psum.tile([128, 128], F32, name="pbn", tag="pmm",
                                    bufs=2)
                    nc.tensor.matmul(pbn[:], BT[:], BN[:], start=True,
                                     stop=True)
                    BN2 = spool.tile([128, 128], BF16, name="BN2", tag="BN2")
                    nc.vector.tensor_copy(BN2[:], pbn[:])
                    if j < 6:
                        pbt = psum.tile([128, 128], F32, name="pbt", tag="pmm",
                                        bufs=2)
                        nc.tensor.matmul(pbt[:], BN[:], BT[:], start=True,
                                         stop=True)
                        BT2 = spool.tile([128, 128], BF16, name="BT2",
                                         tag="BT2")
                        nc.scalar.copy(BT2[:], pbt[:])
                    else:
                        BT2 = None
                    pmt = psum.tile([128, 128], F32, name="pmt", tag="pacc",
                                    bufs=2)
                    nc.tensor.matmul(pmt[:], BN2[:], MTs[:], start=True,
                                     stop=True)
                    MTn = spool.tile([128, 128], BF16, name="MTn", tag="MTs")
                    nc.vector.tensor_add(MTn[:], MTs[:], pmt[:])
                    MTs = MTn
                    BN, BT = BN2, BT2
                TT = MTs

                if STAGE < 6:
                    continue
                # u = T @ (beta v),   wT = -(T @ (beta k))^T
                pu = psum.tile([128, 128], F32, name="pu", tag="pmm", bufs=2)
                nc.tensor.matmul(pu[:], TT[:], vbL[:, cs], start=True,
                                 stop=True)
                uL = dpool.tile([128, 128], BF16, name="uL", tag="uL")
                nc.vector.tensor_copy(uL[:], pu[:])
                pw = psum.tile([128, 128], F32, name="pw", tag="pmm", bufs=2)
                nc.tensor.matmul(pw[:], kbL[:, cs], TT[:], start=True,
                                 stop=True)
                wTs = dpool.tile([128, 128], BF16, name="wTs", tag="wTs")
                nc.vector.tensor_scalar_mul(wTs[:], pw[:], -1.0)

                # ---- sequential scan step ----
                if SUB < 2:
                    continue
                pup = psum.tile([128, 128], F32, name="pup", tag="pmm",
                                bufs=2)
                nc.tensor.matmul(pup[:], wTs[:], Sb[h][:], start=True,
                                 stop=True)
                upb = dpool.tile([128, 128], BF16, name="upb", tag="upb")
                nc.vector.tensor_add(upb[:], uL[:], pup[:])
                if SUB < 3:
                    continue
                po = psum.tile([128, 128], F32, name="po", tag="pacc", bufs=2)
                nc.tensor.matmul(po[:], qT[:, cs], Sb[h][:], start=True,
                                 stop=False)
                nc.tensor.matmul(po[:], attnT[:], upb[:], start=False,
                                 stop=True)
                if SUB < 4:
                    continue
                pds = psum.tile([128, 128], F32, name="pds", tag="pmm",
                                bufs=2)
                nc.tensor.matmul(pds[:], kL[:, cs], upb[:], start=True,
                                 stop=True)
                nc.vector.tensor_add(S[h][:], S[h][:], pds[:])
                nc.scalar.copy(Sb[h][:], S[h][:])

                # ---- gating mix (RMSNorm batched per L-tile below) ----
                if SUB < 5:
                    continue
                og = dpool.tile([128, 128], F32, name="og", tag="og")
                nc.vector.tensor_sub(og[:], po[:], vLb[:, cs])
                og2 = dpool.tile([128, 128], F32, name="og2", tag="og2",
                                 bufs=8)
                nc.vector.scalar_tensor_tensor(og2[:], og[:], g_col,
                                               vLb[:, cs], ALU.mult, ALU.add)
                og2s[(h, cc)] = og2

        # ---- batched per-head RMSNorm + transpose for the 4 chunks ----
        if STAGE < 6 or SUB < 6:
            continue
        ogTs = {}
        for h in range(2):
            ssqb = dpool.tile([128, CPT], F32, name="ssqb", tag="ssqb",
                              bufs=2)
            for cc in range(CPT):
                scr = dpool.tile([128, 128], F32, name="scr", tag="scr")
                nc.scalar.activation(scr[:], og2s[(h, cc)][:], AF.Square,
                                     accum_out=ssqb[:, cc:cc + 1])
            nrb = dpool.tile([128, CPT], F32, name="nrb", tag="nrb", bufs=2)
            nc.scalar.activation(nrb[:], ssqb[:], AF.Sqrt,
                                 bias=eps5[:, 0:1], scale=1.0 / DV)
            nc.vector.reciprocal(nrb[:], nrb[:])
            for cc in range(CPT):
                ogn = dpool.tile([128, 128], F32, name="ogn", tag="ogn")
                nc.vector.scalar_tensor_tensor(
                    ogn[:], og2s[(h, cc)][:], nrb[:, cc:cc + 1], onws[:],
                    ALU.mult, ALU.mult)
                pogt = psum.tile([128, 128], F32, name="pogt", tag="pmm",
                                 bufs=2)
                nc.tensor.transpose(pogt[:], ogn[:], identf[:])
                ogT = dpool.tile([128, 128], F32R, name="ogT", tag="ogT",
                                 bufs=8)
                nc.vector.tensor_copy(ogT[:], pogt[:])
                ogTs[(h, cc)] = ogT

        # ---- output projection (both heads accumulated per chunk) ----
        if STAGE < 7:
            continue
        for cc in range(CPT):
            c = t * CPT + cc
            outb = opool.tile([128, D], F32, name="outb", tag="outb")
            for half in range(2):
                pout = psum.tile([128, 512], F32, name="pout", tag="pout",
                                 bufs=1)
                for h in range(2):
                    nc.tensor.matmul(
                        pout[:], ogTs[(h, cc)][:],
                        wos[:, h * D + half * 512: h * D + (half + 1) * 512],
                        start=(h == 0), stop=(h == 1))
                nc.scalar.copy(outb[:, half * 512:(half + 1) * 512], pout[:])
            nc.sync.dma_start(out[c * 128:(c + 1) * 128, :], outb[:])


_NC_CACHE = None


def _get_program():
    global _NC_CACHE
    if _NC_CACHE is None:
        _NC_CACHE = build_program()
    return _NC_CACHE


def _make_consts():
    bf = ml_dtypes.bfloat16
    ident = np.eye(128, dtype=np.float32)
    return {
        "identf": ident,
        "identb": ident.astype(bf),
        "masksl": (np.tril(np.ones((128, 128), np.float32), -1) * -1.0).astype(bf),
        "maskui": np.triu(np.ones((128, 128), np.float32)).astype(bf),
        "onesc": np.ones((128, 1), np.float32).astype(bf),
        "onesr": np.ones((1, 128), np.float32).astype(bf),
    }


def make_in_maps(inputs):
    hidden = np.asarray(inputs["hidden_states"], np.float32)
    q_w = np.asarray(inputs["q_w"], np.float32)
    k_w = np.asarray(inputs["k_w"], np.float32)
    v_w = np.asarray(inputs["v_w"], np.float32)
    conv_q = np.asarray(inputs["conv_q_w"], np.float32)
    conv_k = np.asarray(inputs["conv_k_w"], np.float32)
    conv_v = np.asarray(inputs["conv_v_w"], np.float32)
    b_w = np.asarray(inputs["b_w"], np.float32)
    mix_w = np.asarray(inputs["mix_w"], np.float32)
    mix_b = np.asarray(inputs["mix_b"], np.float32)
    mix_bias = np.asarray(inputs["mix_bias"], np.float32)
    o_norm_w = np.asarray(inputs["o_norm_w"], np.float32)
    o_w = np.asarray(inputs["o_w"], np.float32)

    consts = _make_consts()
    hT_by_batch = [np.ascontiguousarray(hidden[b].T) for b in range(B)]
    onw_rep = np.ascontiguousarray(np.tile(o_norm_w[None, :], (128, 1)))

    in_maps = []
    for c in range(N_CORES):
        b = c // 4
        h0 = 2 * (c % 4)
        hsl = slice(h0 * DK, (h0 + 2) * DK)
        wbm = np.ascontiguousarray(
            np.stack([b_w[:, h0], b_w[:, h0 + 1],
                      mix_w[:, h0], mix_w[:, h0 + 1]], axis=1))
        bmbias = np.array([[0.0], [0.0],
                           [mix_b[h0] + mix_bias[h0]],
                           [mix_b[h0 + 1] + mix_bias[h0 + 1]]], np.float32)
        m = {
            "hT": hT_by_batch[b],
            "wq": np.ascontiguousarray(q_w[:, hsl]),
            "wk": np.ascontiguousarray(k_w[:, hsl]),
            "wv": np.ascontiguousarray(v_w[:, hsl]),
            "wbm": wbm,
            "bmb": bmbias,
            "cwq": np.ascontiguousarray(conv_q[hsl, :]),
            "cwk": np.ascontiguousarray(conv_k[hsl, :]),
            "cwv": np.ascontiguousarray(conv_v[hsl, :]),
            "wo": np.ascontiguousarray(o_w[hsl, :]),
            "onw": onw_rep,
        }
        m.update(consts)
        in_maps.append(m)
    return in_maps


def kernel(**inputs):
    nc = _get_program()
    in_maps = make_in_maps(inputs)
    res = bass_utils.run_bass_kernel_spmd(nc, in_maps,
                                          core_ids=list(range(N_CORES)))
    outp = np.zeros((B, L, D), np.float32)
    for c in range(N_CORES):
        outp[c // 4] += res.results[c]["out"]
    return outp


# revision 16
# speedup vs baseline: 1.2674x; 1.1019x over previous
"""DeltaNet forward (nn_DeltaNet_31877247271472) as a Trainium2 Bass/Tile kernel.

Sharding: 8 NeuronCores, core c owns batch b=c//4 and heads {2*(c%4), 2*(c%4)+1}.
Each core computes its two heads' full pipeline (projections + causal conv/SiLU +
chunkwise delta rule + gating/RMSNorm + partial output projection); the host sums
the 4 partial outputs per batch.

Per-core algorithm (CHUNK=128, exact restructuring of the reference):
  - projections on the PE in fp32r (hidden pre-transposed to [D, L] on host)
  - delta-rule internals (Gram matrices, UT-transform inverse via the squaring
    series T=(I+B)(I+B^2)...(I+B^64), chunkwise scan) use bf16 operands with
    fp32 accumulation; validated to absmax_rel ~ 8.5e-4 vs the fp32 reference.
  - gating, RMSNorm and the final output projection accumulate in fp32/fp32r.
"""

import sys
from contextlib import ExitStack

try:
    import concourse.bass as bass  # noqa: F401
except ImportError:  # pragma: no cover - environment fallback
    sys.path.insert(0, "/opt/trn_rl_repo")

import numpy as np
import ml_dtypes

import concourse.bass as bass
import concourse.mybir as mybir
import concourse.tile as tile
from concourse import bacc
from concourse import bass_utils

F32 = mybir.dt.float32
F32R = mybir.dt.float32r
BF16 = mybir.dt.bfloat16
AF = mybir.ActivationFunctionType
ALU = mybir.AluOpType

N_CORES = 8
B, L, D, H = 2, 4096, 1024, 8
DK = DV = 128
CONV_K = 4
CHUNK = 128
NCH = L // CHUNK          # 32 chunks per head
KS = D // 128             # 8 contraction slices
LT = 512                  # L-tile (tokens per projection tile)
NLT = L // LT             # 8 L-tiles
CPT = LT // CHUNK         # 4 chunks per L-tile
EPS = 1e-5
STAGE = 99
SUB = 9  # debug bisect
SIM_SAFE = False  # True: emulate SiLU via Sigmoid+mult for CoreSim: 1=proj,2=conv,3=norms,4=transposes,5=gram+series,6=scan+gate,7=outproj


def build_program(n_ltiles=NLT):
    nc = bacc.Bacc("TRN2", target_bir_lowering=False, debug=False,
                   num_devices=N_CORES)

    dt = {}
    def din(name, shape, dtype):
        dt[name] = nc.dram_tensor(name, shape, dtype, kind="ExternalInput").ap()
    din("hT", (D, L), F32R)
    din("wq", (D, 2 * DK), F32R)
    din("wk", (D, 2 * DK), F32R)
    din("wv", (D, 2 * DV), F32R)
    din("wbm", (D, 4), F32R)
    din("bmb", (4, 1), F32)
    din("cwq", (2 * DK, CONV_K), F32)
    din("cwk", (2 * DK, CONV_K), F32)
    din("cwv", (2 * DV, CONV_K), F32)
    din("wo", (2 * DV, D), F32R)
    din("onw", (128, DV), F32)
    din("identf", (128, 128), F32)
    din("identb", (128, 128), BF16)
    din("masksl", (128, 128), BF16)
    din("maskui", (128, 128), BF16)
    din("onesc", (128, 1), BF16)
    din("onesr", (1, 128), BF16)
    out = nc.dram_tensor("out", (L, D), F32, kind="ExternalOutput").ap()

    with tile.TileContext(nc) as tc:
        with ExitStack() as ctx:
            _body(nc, tc, ctx, dt, out, n_ltiles)

    nc.compile()
    return nc


def _body(nc, tc, ctx, dt, out, n_ltiles=NLT):
    cpool = ctx.enter_context(tc.tile_pool(name="consts", bufs=1))
    hpool = ctx.enter_context(tc.tile_pool(name="hts", bufs=2))
    ppool = ctx.enter_context(tc.tile_pool(name="ptmp", bufs=2))
    projp = [ctx.enter_context(tc.tile_pool(name=f"proj{h}", bufs=3))
             for h in range(2)]
    dpool = ctx.enter_context(tc.tile_pool(name="delta", bufs=3))
    spool = ctx.enter_context(tc.tile_pool(name="series", bufs=2))
    opool = ctx.enter_context(tc.tile_pool(name="outb", bufs=2))
    # One PSUM pool; tag budget (1 bank per slot, 8 total):
    #   pp x2, psml x1, pmm x2, pacc x2, pout x1
    psum = ctx.enter_context(tc.tile_pool(name="psum", bufs=1, space="PSUM"))

    # ---- persistent weights/constants -------------------------------------
    wqs = cpool.tile([128, KS * 256], F32R)
    wks = cpool.tile([128, KS * 256], F32R)
    wvs = cpool.tile([128, KS * 256], F32R)
    wbms = cpool.tile([128, KS * 4], F32R)
    bmbias = cpool.tile([4, 1], F32)
    cwt = [[cpool.tile([128, CONV_K], F32, name=f"cw{p}{h}", tag=f"cw{p}{h}")
            for h in range(2)] for p in range(3)]
    wos = cpool.tile([128, 2 * D], F32R)
    onws = cpool.tile([128, DV], F32)
    identf = cpool.tile([128, 128], F32)
    identb = cpool.tile([128, 128], BF16)
    masksl = cpool.tile([128, 128], BF16)
    maskui = cpool.tile([128, 128], BF16)
    onesc = cpool.tile([128, 1], BF16)
    onesr = cpool.tile([1, 128], BF16)
    eps12 = cpool.tile([1, 1], F32)
    nc.vector.memset(eps12[:], 1e-12)
    eps5 = cpool.tile([128, 1], F32)
    nc.vector.memset(eps5[:], EPS)

    for ks in range(KS):
        r = slice(ks * 128, (ks + 1) * 128)
        nc.sync.dma_start(wqs[:, ks * 256:(ks + 1) * 256], dt["wq"][r, :])
        nc.sync.dma_start(wks[:, ks * 256:(ks + 1) * 256], dt["wk"][r, :])
        nc.sync.dma_start(wvs[:, ks * 256:(ks + 1) * 256], dt["wv"][r, :])
        nc.sync.dma_start(wbms[:, ks * 4:(ks + 1) * 4], dt["wbm"][r, :])
    nc.sync.dma_start(bmbias[:], dt["bmb"][:])
    for p, cwn in enumerate(("cwq", "cwk", "cwv")):
        for h in range(2):
            nc.sync.dma_start(cwt[p][h][:], dt[cwn][h * 128:(h + 1) * 128, :])
    for h in range(2):
        nc.sync.dma_start(wos[:, h * D:(h + 1) * D],
                          dt["wo"][h * 128:(h + 1) * 128, :])
    for n, t_ in (("onw", onws), ("identf", identf), ("identb", identb),
                  ("masksl", masksl), ("maskui", maskui), ("onesc", onesc),
                  ("onesr", onesr)):
        nc.sync.dma_start(t_[:], dt[n][:])

    # ---- loop-carried state ------------------------------------------------
    S = [cpool.tile([DK, DV], F32, name=f"S{h}", tag=f"S{h}") for h in range(2)]
    Sb = [cpool.tile([DK, DV], BF16, name=f"Sb{h}", tag=f"Sb{h}")
          for h in range(2)]
    for h in range(2):
        nc.vector.memset(S[h][:], 0.0)
        nc.vector.memset(Sb[h][:], 0.0)

    tails = {}
    for p in range(3):
        for h in range(2):
            tl = cpool.tile([128, CONV_K - 1], BF16, name=f"tl{p}{h}",
                            tag=f"tail{p}{h}", bufs=2)
            nc.vector.memset(tl[:], 0.0)
            tails[(p, h)] = tl

    win = [{} for _ in range(2)]

    for t in range(n_ltiles):
        lo = t * LT
        # ================= phase P: projections for L-tile t ================
        hts = []
        for ks in range(KS):
            ht = hpool.tile([128, LT], F32R, name=f"ht{ks}", tag=f"ht{ks}",
                            bufs=2)
            nc.sync.dma_start(ht[:], dt["hT"][ks * 128:(ks + 1) * 128,
                                              lo:lo + LT])
            hts.append(ht)

        # beta/mix projection (uses the single small psum slot first)
        psbm = psum.tile([4, LT], F32, name="psbm", tag="psml", bufs=1)
        for ks in range(KS):
            nc.tensor.matmul(psbm[:], wbms[:, ks * 4:(ks + 1) * 4], hts[ks][:],
                             start=(ks == 0), stop=(ks == KS - 1))
        bmT = ppool.tile([4, LT], F32, name="bmT", tag="bmT", bufs=2)
        nc.scalar.activation(bmT[:], psbm[:], AF.Sigmoid, bias=bmbias[:, 0:1])
        bmL = ppool.tile([128, CPT * 4], F32, name="bmL", tag="bmL", bufs=2)
        for st in range(CPT):
            pt = psum.tile([128, 4], F32, name="pbmt", tag="psml", bufs=1)
            nc.tensor.transpose(pt[:], bmT[0:4, st * 128:(st + 1) * 128],
                                identf[0:4, 0:4])
            nc.vector.tensor_copy(bmL[:, st * 4:(st + 1) * 4], pt[:])

        # q/k/v projections for both heads: 3 passes of 2 psum banks
        ys_tiles = {}
        for p, wsb in enumerate((wqs, wks, wvs)):
            for h in range(2):
                ps = psum.tile([128, LT], F32, name=f"pp{p}{h}", tag="pp",
                               bufs=2)
                for ks in range(KS):
                    nc.tensor.matmul(
                        ps[:],
                        wsb[:, ks * 256 + h * 128: ks * 256 + (h + 1) * 128],
                        hts[ks][:], start=(ks == 0), stop=(ks == KS - 1))
                # causal depthwise conv (K=4) + SiLU
                if STAGE < 2:
                    ys_tiles[(p, h)] = None
                    continue
                xcat = ppool.tile([128, LT + CONV_K - 1], BF16, name="xcat",
                                  tag="xcat", bufs=2)
                nc.scalar.copy(xcat[:, CONV_K - 1:], ps[:])
                nc.vector.tensor_copy(xcat[:, 0:CONV_K - 1], tails[(p, h)][:])
                ntail = cpool.tile([128, CONV_K - 1], BF16, name=f"ntl{p}{h}",
                                   tag=f"tail{p}{h}", bufs=2)
                nc.vector.tensor_copy(ntail[:], xcat[:, LT:LT + CONV_K - 1])
                tails[(p, h)] = ntail
                cw = cwt[p][h]
                ya = ppool.tile([128, LT], F32, name="ya", tag="ya", bufs=2)
                yb = ppool.tile([128, LT], F32, name="yb", tag="yb", bufs=2)
                nc.vector.tensor_scalar_mul(ya[:], xcat[:, 3:3 + LT],
                                            cw[:, 3:4])
                nc.vector.scalar_tensor_tensor(
                    yb[:], xcat[:, 2:2 + LT], cw[:, 2:3], ya[:],
                    ALU.mult, ALU.add)
                nc.vector.scalar_tensor_tensor(
                    ya[:], xcat[:, 1:1 + LT], cw[:, 1:2], yb[:],
                    ALU.mult, ALU.add)
                nc.vector.scalar_tensor_tensor(
                    yb[:], xcat[:, 0:0 + LT], cw[:, 0:1], ya[:],
                    ALU.mult, ALU.add)
                ys = ppool.tile([128, LT], F32, name=f"ys{p}{h}",
                                tag=f"ys{p}{h}", bufs=2)
                if SIM_SAFE:
                    # CoreSim has no Silu table; emulate
                    sgm = ppool.tile([128, LT], F32, name="sgm", tag="sgm",
                                     bufs=2)
                    nc.scalar.activation(sgm[:], yb[:], AF.Sigmoid)
                    nc.vector.tensor_mul(ys[:], yb[:], sgm[:])
                else:
                    nc.scalar.activation(ys[:], yb[:], AF.Silu)
                ys_tiles[(p, h)] = ys

        if STAGE < 3:
            continue
        for h in range(2):
            w = win[h]
            # q,k: token-wise l2 norm via column-sum matmul trick (d-major)
            for p, name in ((0, "qTb"), (1, "kTb")):
                ys = ys_tiles[(p, h)]
                sq = ppool.tile([128, LT], BF16, name="sqt", tag="sqt", bufs=2)
                nc.vector.tensor_mul(sq[:], ys[:], ys[:])
                prow = psum.tile([1, LT], F32, name="prow", tag="psml", bufs=1)
                nc.tensor.matmul(prow[:], onesc[:], sq[:], start=True,
                                 stop=True)
                rrowb = ppool.tile([1, LT], BF16, name="rrowb", tag="rrowb",
                                   bufs=2)
                if SIM_SAFE:
                    rrow = ppool.tile([1, LT], F32, name="rrow", tag="rrow",
                                      bufs=2)
                    nc.scalar.activation(rrow[:], prow[:], AF.Sqrt,
                                         bias=eps12[:, 0:1])
                    nc.vector.reciprocal(rrow[:], rrow[:])
                    nc.vector.tensor_copy(rrowb[:], rrow[:])
                else:
                    nc.scalar.activation(rrowb[:], prow[:],
                                         AF.Abs_reciprocal_sqrt,
                                         bias=eps12[:, 0:1])
                prep = psum.tile([128, LT], F32, name="prep", tag="psml",
                                 bufs=1)
                nc.tensor.matmul(prep[:], onesr[:], rrowb[:], start=True,
                                 stop=True)
                nT = projp[h].tile([128, LT], BF16, name=name, tag=name)
                nc.vector.tensor_mul(nT[:], ys[:], prep[:])
                w[(name, t)] = nT

            if STAGE < 4:
                continue
            # k -> l-major (normalized) and kb = beta*k
            kT = w[("kTb", t)]
            kL = projp[h].tile([128, LT], BF16, name="kL", tag="kL")
            kbL = projp[h].tile([128, LT], BF16, name="kbL", tag="kbL")
            for st in range(CPT):
                ptr = psum.tile([128, 128], BF16, name="ptr", tag="psml",
                                bufs=1)
                nc.tensor.transpose(ptr[:], kT[:, st * 128:(st + 1) * 128],
                                    identb[:])
                nc.vector.tensor_copy(kL[:, st * 128:(st + 1) * 128], ptr[:])
                nc.vector.tensor_scalar_mul(
                    kbL[:, st * 128:(st + 1) * 128], ptr[:],
                    bmL[:, st * 4 + h: st * 4 + h + 1])
            w[("kL", t)] = kL
            w[("kbL", t)] = kbL

            # v -> l-major, vb = beta*v
            vs = ys_tiles[(2, h)]
            vLb = projp[h].tile([128, LT], BF16, name="vLb", tag="vLb")
            vbL = projp[h].tile([128, LT], BF16, name="vbL", tag="vbL")
            for st in range(CPT):
                ptr = psum.tile([128, 128], F32, name="ptrv", tag="psml",
                                bufs=1)
                nc.tensor.transpose(ptr[:], vs[:, st * 128:(st + 1) * 128],
                                    identf[:])
                nc.vector.tensor_copy(vLb[:, st * 128:(st + 1) * 128], ptr[:])
                nc.vector.tensor_scalar_mul(
                    vbL[:, st * 128:(st + 1) * 128], ptr[:],
                    bmL[:, st * 4 + h: st * 4 + h + 1])
            w[("vLb", t)] = vLb
            w[("vbL", t)] = vbL
            w[("bmL", t)] = bmL

        # ================= phase D: delta rule, 4 chunks ====================
        if STAGE < 5:
            continue
        og2s = {}
        for cc in range(CPT):
            c = t * CPT + cc
            cs = slice(cc * 128, (cc + 1) * 128)
            for h in range(2):
                w = win[h]
                qT, kT = w[("qTb", t)], w[("kTb", t)]
                kL, kbL = w[("kL", t)], w[("kbL", t)]
                vLb, vbL = w[("vLb", t)], w[("vbL", t)]
                bml = w[("bmL", t)]
                beta_col = bml[:, cc * 4 + h: cc * 4 + h + 1]
                g_col = bml[:, cc * 4 + 2 + h: cc * 4 + 3 + h]

                # Gram matrices
                pg = psum.tile([128, 128], F32, name="pg", tag="pmm", bufs=2)
                nc.tensor.matmul(pg[:], kT[:, cs], kT[:, cs], start=True,
                                 stop=True)
                pa = psum.tile([128, 128], F32, name="pa", tag="pmm", bufs=2)
                nc.tensor.matmul(pa[:], kT[:, cs], qT[:, cs], start=True,
                                 stop=True)
                AN = dpool.tile([128, 128], BF16, name="AN", tag="AN")
                nc.vector.scalar_tensor_tensor(AN[:], pg[:], beta_col,
                                               masksl[:], ALU.mult, ALU.mult)
                attnT = dpool.tile([128, 128], BF16, name="attnT", tag="attnT")
                nc.vector.tensor_mul(attnT[:], pa[:], maskui[:])
                pat = psum.tile([128, 128], BF16, name="pat", tag="pmm",
                                bufs=2)
                nc.tensor.transpose(pat[:], AN[:], identb[:])
                ATl = dpool.tile([128, 128], BF16, name="ATl", tag="ATl")
                nc.vector.tensor_copy(ATl[:], pat[:])

                # UT-transform inverse via squaring series (bf16 operands)
                MTs = spool.tile([128, 128], BF16, name="MTs", tag="MTs")
                nc.vector.tensor_add(MTs[:], ATl[:], identb[:])
                BN, BT = AN, ATl
                for j in range(1, 7):
                    pbn = psum.tile([128, 128], F32, name="pbn", tag="pmm",
                                    bufs=2)
                    nc.tensor.matmul(pbn[:], BT[:], BN[:], start=True,
                                     stop=True)
                    BN2 = spool.tile([128, 128], BF16, name="BN2", tag="BN2")
                    nc.vector.tensor_copy(BN2[:], pbn[:])
                    if j < 6:
                        pbt = psum.tile([128, 128], F32, name="pbt", tag="pmm",
                                        bufs=2)
                        nc.tensor.matmul(pbt[:], BN[:], BT[:], start=True,
                                         stop=True)
                        BT2 = spool.tile([128, 128], BF16, name="BT2",
                                         tag="BT2")
                        nc.scalar.copy(BT2[:], pbt[:])
                    else:
                        BT2 = None
                    pmt = psum.tile([128, 128], F32, name="pmt", tag="pacc",
                                    bufs=2)
                    nc.tensor.matmul(pmt[:], BN2[:], MTs[:], start=True,
                                     stop=True)
                    MTn = spool.tile([128, 128], BF16, name="MTn", tag="MTs")
                    nc.vector.tensor_add(MTn[:], MTs[:], pmt[:])
                    MTs = MTn
                    BN, BT = BN2, BT2
                TT = MTs

                if STAGE < 6:
                    continue
                # u = T @ (beta v),   wT = -(T @ (beta k))^T
                pu = psum.tile([128, 128], F32, name="pu", tag="pmm", bufs=2)
                nc.tensor.matmul(pu[:], TT[:], vbL[:, cs], start=True,
                                 stop=True)
                uL = dpool.tile([128, 128], BF16, name="uL", tag="uL")
                nc.vector.tensor_copy(uL[:], pu[:])
                pw = psum.tile([128, 128], F32, name="pw", tag="pmm", bufs=2)
                nc.tensor.matmul(pw[:], kbL[:, cs], TT[:], start=True,
                                 stop=True)
                wTs = dpool.tile([128, 128], BF16, name="wTs", tag="wTs")
                nc.vector.tensor_scalar_mul(wTs[:], pw[:], -1.0)

                # ---- sequential scan step ----
                if SUB < 2:
                    continue
                pup = psum.tile([128, 128], F32, name="pup", tag="pmm",
                                bufs=2)
                nc.tensor.matmul(pup[:], wTs[:], Sb[h][:], start=True,
                                 stop=True)
                upb = dpool.tile([128, 128], BF16, name="upb", tag="upb")
                nc.vector.tensor_add(upb[:], uL[:], pup[:])
                if SUB < 3:
                    continue
                po = psum.tile([128, 128], F32, name="po", tag="pacc", bufs=2)
                nc.tensor.matmul(po[:], qT[:, cs], Sb[h][:], start=True,
                                 stop=False)
                nc.tensor.matmul(po[:], attnT[:], upb[:], start=False,
                                 stop=True)
                if SUB < 4:
                    continue
                pds = psum.tile([128, 128], F32, name="pds", tag="pmm",
                                bufs=2)
                nc.tensor.matmul(pds[:], kL[:, cs], upb[:], start=True,
                                 stop=True)
                nc.vector.tensor_add(S[h][:], S[h][:], pds[:])
                nc.scalar.copy(Sb[h][:], S[h][:])

                # ---- gating mix (RMSNorm batched per L-tile below) ----
                if SUB < 5:
                    continue
                og = dpool.tile([128, 128], F32, name="og", tag="og")
                nc.vector.tensor_sub(og[:], po[:], vLb[:, cs])
                og2 = dpool.tile([128, 128], F32, name="og2", tag="og2",
                                 bufs=8)
                nc.vector.scalar_tensor_tensor(og2[:], og[:], g_col,
                                               vLb[:, cs], ALU.mult, ALU.add)
                og2s[(h, cc)] = og2

        # ---- batched per-head RMSNorm + transpose for the 4 chunks ----
        if STAGE < 6 or SUB < 6:
            continue
        ogTs = {}
        for h in range(2):
            ssqb = dpool.tile([128, CPT], F32, name="ssqb", tag="ssqb",
                              bufs=2)
            for cc in range(CPT):
                scr = dpool.tile([128, 128], F32, name="scr", tag="scr")
                nc.scalar.activation(scr[:], og2s[(h, cc)][:], AF.Square,
                                     accum_out=ssqb[:, cc:cc + 1])
            nrb = dpool.tile([128, CPT], F32, name="nrb", tag="nrb", bufs=2)
            if SIM_SAFE:
                nc.scalar.activation(nrb[:], ssqb[:], AF.Sqrt,
                                     bias=eps5[:, 0:1], scale=1.0 / DV)
                nc.vector.reciprocal(nrb[:], nrb[:])
            else:
                nc.scalar.activation(nrb[:], ssqb[:], AF.Abs_reciprocal_sqrt,
                                     bias=eps5[:, 0:1], scale=1.0 / DV)
            for cc in range(CPT):
                ogn = dpool.tile([128, 128], F32, name="ogn", tag="ogn")
                nc.vector.scalar_tensor_tensor(
                    ogn[:], og2s[(h, cc)][:], nrb[:, cc:cc + 1], onws[:],
                    ALU.mult, ALU.mult)
                pogt = psum.tile([128, 128], F32, name="pogt", tag="pmm",
                                 bufs=2)
                nc.tensor.transpose(pogt[:], ogn[:], identf[:])
                ogT = dpool.tile([128, 128], F32R, name="ogT", tag="ogT",
                                 bufs=8)
                nc.vector.tensor_copy(ogT[:], pogt[:])
                ogTs[(h, cc)] = ogT

        # ---- output projection (both heads accumulated per chunk) ----
        if STAGE < 7:
            continue
        for cc in range(CPT):
            c = t * CPT + cc
            outb = opool.tile([128, D], F32, name="outb", tag="outb")
            for half in range(2):
                pout = psum.tile([128, 512], F32, name="pout", tag="pout",
                                 bufs=1)
                for h in range(2):
                    nc.tensor.matmul(
                        pout[:], ogTs[(h, cc)][:],
                        wos[:, h * D + half * 512: h * D + (half + 1) * 512],
                        start=(h == 0), stop=(h == 1))
                nc.scalar.copy(outb[:, half * 512:(half + 1) * 512], pout[:])
            nc.sync.dma_start(out[c * 128:(c + 1) * 128, :], outb[:])


_NC_CACHE = None


def _get_program():
    global _NC_CACHE
    if _NC_CACHE is None:
        _NC_CACHE = build_program()
    return _NC_CACHE


def _make_consts():
    bf = ml_dtypes.bfloat16
    ident = np.eye(128, dtype=np.float32)
    return {
        "identf": ident,
        "identb": ident.astype(bf),
        "masksl": (np.tril(np.ones((128, 128), np.float32), -1) * -1.0).astype(bf),
        "maskui": np.triu(np.ones((128, 128), np.float32)).astype(bf),
        "onesc": np.ones((128, 1), np.float32).astype(bf),
        "onesr": np.ones((1, 128), np.float32).astype(bf),
    }


def make_in_maps(inputs):
    hidden = np.asarray(inputs["hidden_states"], np.float32)
    q_w = np.asarray(inputs["q_w"], np.float32)
    k_w = np.asarray(inputs["k_w"], np.float32)
    v_w = np.asarray(inputs["v_w"], np.float32)
    conv_q = np.asarray(inputs["conv_q_w"], np.float32)
    conv_k = np.asarray(inputs["conv_k_w"], np.float32)
    conv_v = np.asarray(inputs["conv_v_w"], np.float32)
    b_w = np.asarray(inputs["b_w"], np.float32)
    mix_w = np.asarray(inputs["mix_w"], np.float32)
    mix_b = np.asarray(inputs["mix_b"], np.float32)
    mix_bias = np.asarray(inputs["mix_bias"], np.float32)
    o_norm_w = np.asarray(inputs["o_norm_w"], np.float32)
    o_w = np.asarray(inputs["o_w"], np.float32)

    consts = _make_consts()
    hT_by_batch = [np.ascontiguousarray(hidden[b].T) for b in range(B)]
    onw_rep = np.ascontiguousarray(np.tile(o_norm_w[None, :], (128, 1)))

    in_maps = []
    for c in range(N_CORES):
        b = c // 4
        h0 = 2 * (c % 4)
        hsl = slice(h0 * DK, (h0 + 2) * DK)
        wbm = np.ascontiguousarray(
            np.stack([b_w[:, h0], b_w[:, h0 + 1],
                      mix_w[:, h0], mix_w[:, h0 + 1]], axis=1))
        bmbias = np.array([[0.0], [0.0],
                           [mix_b[h0] + mix_bias[h0]],
                           [mix_b[h0 + 1] + mix_bias[h0 + 1]]], np.float32)
        m = {
            "hT": hT_by_batch[b],
            "wq": np.ascontiguousarray(q_w[:, hsl]),
            "wk": np.ascontiguousarray(k_w[:, hsl]),
            "wv": np.ascontiguousarray(v_w[:, hsl]),
            "wbm": wbm,
            "bmb": bmbias,
            "cwq": np.ascontiguousarray(conv_q[hsl, :]),
            "cwk": np.ascontiguousarray(conv_k[hsl, :]),
            "cwv": np.ascontiguousarray(conv_v[hsl, :]),
            "wo": np.ascontiguousarray(o_w[hsl, :]),
            "onw": onw_rep,
        }
        m.update(consts)
        in_maps.append(m)
    return in_maps


def kernel(**inputs):
    nc = _get_program()
    in_maps = make_in_maps(inputs)
    res = bass_utils.run_bass_kernel_spmd(nc, in_maps,
                                          core_ids=list(range(N_CORES)))
    outp = np.zeros((B, L, D), np.float32)
    for c in range(N_CORES):
        outp[c // 4] += res.results[c]["out"]
    return outp
